# revision 10
# baseline (speedup 1.0000x reference)
"""ComplexConv2D Trainium2 kernel.

Reference computation (B=16, H=W=128, CIN=64, F=128, K=3, SAME, stride 1):
    real_out = conv(x_real, k_real) - conv(x_imag, k_imag) + b_real
    imag_out = conv(x_real, k_imag) + conv(x_imag, k_real) + b_imag
    return stack([real_out, imag_out])           # [2, B, H, W, F]

Strategy:
  * Data-parallel over batch: 2 images per NeuronCore x 8 cores.
  * Complex arithmetic is folded into the matmul contraction: stack
    [x_real; x_imag] channel-wise (K = 2*CIN = 128 = full PE width) and
    contract against stacked weights [k_real; -k_imag] (real part) and
    [k_imag; k_real] (imag part).  Each output part is then ONE ordinary
    3x3 conv with 128 input channels.
  * The conv is 9 shifted matmuls accumulated in PSUM.  The image lives in
    SBUF channel-major as [128ch, (H+4)*(W+2)] with a 1-pixel zero border;
    a tap (dy,dx) is just a free-dim slice offset dy*(W+2)+dx, so all 9
    taps stream from the same SBUF buffer with zero data movement.
  * Matmul: lhsT = weight tap [128ch, 128F] (stationary), rhs = image
    slice [128ch, 512pos] (moving), PSUM tile [128F, 512pos] fp32 = one
    bank.  9 accumulating matmuls per tile; 33 tiles cover one image.
  * bf16 inputs (host-cast), fp32 PSUM accumulation, fp32 output.
  * Output leaves the chip channel-major [F, positions]; the final
    transpose to NHWC plus removal of the 2 pad columns per row happens
    host-side during the gather.
"""

import os

import numpy as np
import ml_dtypes

B, H, W, CIN, F = 16, 128, 128, 64, 128
N_CORES = 8
B_LOC = B // N_CORES          # images per core
RS = W + 2                    # padded row stride (130)
ROWS = H + 3                  # 1 top pad + 1 bottom pad + 1 slack row (131)
L = ROWS * RS                 # flat padded image length (17030)
NQ_VALID = H * RS             # flat positions covering all valid outputs (16640)
NT = 33                       # output tiles per image: 32x512 + 1x256
NQ = NQ_VALID                 # flat output length on chip (16640)
TILE_N = [512] * 32 + [256]
TILE_Q0 = [512 * t for t in range(33)]

_BF16 = ml_dtypes.bfloat16

_CACHE = {}


def _legalize_single_wait(nc):
    """The pinned walrus build in this container accepts only a single
    sync-wait per instruction.  Tile attaches several waits to one
    instruction (drain, DMA, matmul...).  Hoist all-but-one wait onto
    fresh no-fuse NoOps on the same engine placed immediately before the
    instruction — same-engine program order preserves the AND semantics."""
    import concourse.mybir as mybir

    for f in nc.m.functions:
        for bb in f.blocks:
            newlist = []
            for inst in bb.instructions:
                si = inst.sync_info
                if si is not None and len(si.on_wait) > 1:
                    waits = list(si.on_wait)
                    del si.on_wait[:]
                    si.on_wait.append(waits[-1])
                    for k, w in enumerate(waits[:-1]):
                        nop = mybir.InstNoOp(
                            name=f"{inst.name}.sw{k}",
                            opcode="NoOp",
                            engine=inst.engine,
                            bass_nofuse=True,
                            sync_info=mybir.SyncInfo(on_wait=[w], on_update=[]),
                        )
                        newlist.append(nop)
                newlist.append(inst)
            bb.instructions[:] = newlist


def _build_nc():
    import concourse.bass as bass
    import concourse.mybir as mybir
    import concourse.tile as tile

    nc = bass.Bass()
    xs = nc.declare_dram_parameter("xs", [B_LOC, 128, L], mybir.dt.bfloat16, isOutput=False)
    kw = nc.declare_dram_parameter("kw", [128, 2 * 9 * F], mybir.dt.bfloat16, isOutput=False)
    out = nc.declare_dram_parameter("out", [2, B_LOC, F, NQ], mybir.dt.float32, isOutput=True)

    # graduated chunk boundaries: small first chunks so the first matmuls
    # can start as soon as ~0.3 MB has landed, big chunks for efficiency
    CHUNKS = [0, 1040, 3120, 6500, 10010, 13520, L]

    with tile.TileContext(nc) as tc:
        with (
            tc.tile_pool(name="kw", bufs=1) as kwp,
            tc.tile_pool(name="img", bufs=2) as imgp,
            tc.tile_pool(name="psum", bufs=8, space="PSUM") as psp,
            tc.tile_pool(name="stage", bufs=4) as stp,
        ):
            kw_sb = kwp.tile([128, 2 * 9 * F], mybir.dt.bfloat16)
            # The first LDWEIGHTS gates the whole pipeline: load the first
            # three taps alone (96 KB) so their completion isn't delayed by
            # image traffic, then the rest of part 0; part-1 weights aren't
            # needed until ~half way through image 0.
            nc.sync.dma_start(out=kw_sb[:, :3 * F], in_=kw[:, :3 * F])
            nc.sync.dma_start(out=kw_sb[:, 3 * F:9 * F], in_=kw[:, 3 * F:9 * F])

            first = True
            for b in range(B_LOC):
                img = imgp.tile([128, L], mybir.dt.bfloat16)
                for c0, c1 in zip(CHUNKS, CHUNKS[1:]):
                    nc.sync.dma_start(out=img[:, c0:c1], in_=xs[b, :, c0:c1])
                if first:
                    nc.sync.dma_start(out=kw_sb[:, 9 * F:], in_=kw[:, 9 * F:])
                    first = False
                for part in range(2):
                    for t in range(NT):
                        q0, n = TILE_Q0[t], TILE_N[t]
                        ps = psp.tile([128, 512], mybir.dt.float32)
                        for tap in range(9):
                            dy, dx = divmod(tap, 3)
                            off = q0 + dy * RS + dx
                            nc.tensor.matmul(
                                ps[:, :n],
                                kw_sb[:, (part * 9 + tap) * F:(part * 9 + tap + 1) * F],
                                img[:, off:off + n],
                                start=(tap == 0),
                                stop=(tap == 8),
                            )
                        st = stp.tile([128, 512], mybir.dt.float32)
                        nc.vector.tensor_copy(st[:, :n], ps[:, :n])
                        nc.sync.dma_start(out=out[part, b, :, q0:q0 + n], in_=st[:, :n])

    _legalize_single_wait(nc)
    return nc


LAST_RESULT = None


def _ensure_axon_hooks_stub():
    """bass_utils imports antenv.axon_hooks when BASS_TRACE is set; the
    module is absent from this image.  Provide a no-op stub (get -> None)
    unless something already registered a real hook."""
    import sys
    import types

    if "antenv.axon_hooks" in sys.modules:
        return
    mod = types.ModuleType("antenv.axon_hooks")
    mod._hook = None
    mod.set_axon_ntff_profile_hook = lambda h: setattr(mod, "_hook", h)
    mod.get_axon_ntff_profile_hook = lambda: mod._hook
    sys.modules["antenv.axon_hooks"] = mod


def kernel(x_real, x_imag, k_real, k_imag, b_real, b_imag):
    global LAST_RESULT
    _ensure_axon_hooks_stub()
    from concourse.bass_utils import run_bass_kernel_spmd

    x_real = np.asarray(x_real, dtype=np.float32)
    x_imag = np.asarray(x_imag, dtype=np.float32)
    k_real = np.asarray(k_real, dtype=np.float32)
    k_imag = np.asarray(k_imag, dtype=np.float32)
    b_real = np.asarray(b_real, dtype=np.float32)
    b_imag = np.asarray(b_imag, dtype=np.float32)

    # ---- host-side input prep -------------------------------------------
    # padded channel-major image, channels = [x_real; x_imag]
    xp = np.zeros((B, ROWS, RS, 2 * CIN), np.float32)
    xp[:, 1:H + 1, 1:W + 1, :CIN] = x_real
    xp[:, 1:H + 1, 1:W + 1, CIN:] = x_imag
    xs_all = np.ascontiguousarray(xp.transpose(0, 3, 1, 2).reshape(B, 128, L)).astype(_BF16)

    # stacked weights: [ch, part, dy, dx, F] -> [128, 2304]
    wr = np.concatenate([k_real, -k_imag], axis=2)   # [3,3,128,F]
    wi = np.concatenate([k_imag, k_real], axis=2)
    kw = np.ascontiguousarray(
        np.stack([wr, wi]).transpose(3, 0, 1, 2, 4).reshape(128, 2 * 9 * F)
    ).astype(_BF16)

    if "nc" not in _CACHE:
        _CACHE["nc"] = _build_nc()
    nc = _CACHE["nc"]

    in_maps = [
        {"xs": xs_all[c * B_LOC:(c + 1) * B_LOC], "kw": kw} for c in range(N_CORES)
    ]
    res = run_bass_kernel_spmd(nc, in_maps, core_ids=list(range(N_CORES)))
    LAST_RESULT = res

    # ---- host-side gather / unshard -------------------------------------
    final = np.empty((2, B, H, W, F), np.float32)
    for c in range(N_CORES):
        oc = res.results[c]["out"]                       # [2, B_LOC, F, NQ]
        v = oc.reshape(2, B_LOC, F, H, RS)[..., :W]
        final[:, c * B_LOC:(c + 1) * B_LOC] = v.transpose(0, 1, 3, 4, 2)

    if b_real.any():
        final[0] += b_real
    if b_imag.any():
        final[1] += b_imag
    return final


# revision 12
# speedup vs baseline: 1.0010x; 1.0010x over previous
"""ComplexConv2D Trainium2 kernel.

Reference computation (B=16, H=W=128, CIN=64, F=128, K=3, SAME, stride 1):
    real_out = conv(x_real, k_real) - conv(x_imag, k_imag) + b_real
    imag_out = conv(x_real, k_imag) + conv(x_imag, k_real) + b_imag
    return stack([real_out, imag_out])           # [2, B, H, W, F]

Strategy:
  * Data-parallel over batch: 2 images per NeuronCore x 8 cores.
  * Complex arithmetic is folded into the matmul contraction: stack
    [x_real; x_imag] channel-wise (K = 2*CIN = 128 = full PE width) and
    contract against stacked weights [k_real; -k_imag] (real part) and
    [k_imag; k_real] (imag part).  Each output part is then ONE ordinary
    3x3 conv with 128 input channels.
  * The conv is 9 shifted matmuls accumulated in PSUM.  The image lives in
    SBUF channel-major as [128ch, (H+4)*(W+2)] with a 1-pixel zero border;
    a tap (dy,dx) is just a free-dim slice offset dy*(W+2)+dx, so all 9
    taps stream from the same SBUF buffer with zero data movement.
  * Matmul: lhsT = weight tap [128ch, 128F] (stationary), rhs = image
    slice [128ch, 512pos] (moving), PSUM tile [128F, 512pos] fp32 = one
    bank.  9 accumulating matmuls per tile; 33 tiles cover one image.
  * bf16 inputs (host-cast), fp32 PSUM accumulation, fp32 output.
  * Output leaves the chip channel-major [F, positions]; the final
    transpose to NHWC plus removal of the 2 pad columns per row happens
    host-side during the gather.
"""

import os

import numpy as np
import ml_dtypes

B, H, W, CIN, F = 16, 128, 128, 64, 128
N_CORES = 8
B_LOC = B // N_CORES          # images per core
RS = W + 2                    # padded row stride (130)
ROWS = H + 3                  # 1 top pad + 1 bottom pad + 1 slack row (131)
L = ROWS * RS                 # flat padded image length (17030)
NQ_VALID = H * RS             # flat positions covering all valid outputs (16640)
NT = 33                       # output tiles per image: 32x512 + 1x256
NQ = NQ_VALID                 # flat output length on chip (16640)
TILE_N = [512] * 32 + [256]
TILE_Q0 = [512 * t for t in range(33)]

_BF16 = ml_dtypes.bfloat16

_CACHE = {}


def _legalize_single_wait(nc):
    """The pinned walrus build in this container accepts only a single
    sync-wait per instruction.  Tile attaches several waits to one
    instruction (drain, DMA, matmul...).  Hoist all-but-one wait onto
    fresh no-fuse NoOps on the same engine placed immediately before the
    instruction — same-engine program order preserves the AND semantics."""
    import concourse.mybir as mybir

    for f in nc.m.functions:
        for bb in f.blocks:
            newlist = []
            for inst in bb.instructions:
                si = inst.sync_info
                if si is not None and len(si.on_wait) > 1:
                    waits = list(si.on_wait)
                    del si.on_wait[:]
                    si.on_wait.append(waits[-1])
                    for k, w in enumerate(waits[:-1]):
                        nop = mybir.InstNoOp(
                            name=f"{inst.name}.sw{k}",
                            opcode="NoOp",
                            engine=inst.engine,
                            bass_nofuse=True,
                            sync_info=mybir.SyncInfo(on_wait=[w], on_update=[]),
                        )
                        newlist.append(nop)
                newlist.append(inst)
            bb.instructions[:] = newlist


def _build_nc():
    import concourse.bass as bass
    import concourse.mybir as mybir
    import concourse.tile as tile

    nc = bass.Bass()
    xs = nc.declare_dram_parameter("xs", [B_LOC, 128, L], mybir.dt.bfloat16, isOutput=False)
    kw = nc.declare_dram_parameter("kw", [128, 2 * 9 * F], mybir.dt.bfloat16, isOutput=False)
    out = nc.declare_dram_parameter("out", [2, B_LOC, F, NQ], mybir.dt.float32, isOutput=True)

    # graduated chunk boundaries: small first chunks so the first matmuls
    # can start as soon as ~0.3 MB has landed, big chunks for efficiency
    CHUNKS = [0, 1040, 3120, 6500, 10010, 13520, L]

    with tile.TileContext(nc) as tc:
        with (
            tc.tile_pool(name="kw", bufs=1) as kwp,
            tc.tile_pool(name="img", bufs=2) as imgp,
            tc.tile_pool(name="psum", bufs=8, space="PSUM") as psp,
            tc.tile_pool(name="stage", bufs=4) as stp,
        ):
            kw_sb = kwp.tile([128, 2 * 9 * F], mybir.dt.bfloat16)
            # The first LDWEIGHTS gates the whole pipeline: load the first
            # three taps alone (96 KB) so their completion isn't delayed by
            # image traffic, then the rest of part 0; part-1 weights aren't
            # needed until ~half way through image 0.
            nc.sync.dma_start(out=kw_sb[:, :3 * F], in_=kw[:, :3 * F])

            first = True
            for b in range(B_LOC):
                img = imgp.tile([128, L], mybir.dt.bfloat16)
                for ci, (c0, c1) in enumerate(zip(CHUNKS, CHUNKS[1:])):
                    nc.sync.dma_start(out=img[:, c0:c1], in_=xs[b, :, c0:c1])
                    if first and ci == 0:
                        # rest of part-0 weights ride behind the first chunk
                        nc.sync.dma_start(out=kw_sb[:, 3 * F:9 * F], in_=kw[:, 3 * F:9 * F])
                if first:
                    nc.sync.dma_start(out=kw_sb[:, 9 * F:], in_=kw[:, 9 * F:])
                    first = False
                for part in range(2):
                    for t in range(NT):
                        q0, n = TILE_Q0[t], TILE_N[t]
                        ps = psp.tile([128, 512], mybir.dt.float32)
                        for tap in range(9):
                            dy, dx = divmod(tap, 3)
                            off = q0 + dy * RS + dx
                            nc.tensor.matmul(
                                ps[:, :n],
                                kw_sb[:, (part * 9 + tap) * F:(part * 9 + tap + 1) * F],
                                img[:, off:off + n],
                                start=(tap == 0),
                                stop=(tap == 8),
                            )
                        st = stp.tile([128, 512], mybir.dt.float32)
                        nc.vector.tensor_copy(st[:, :n], ps[:, :n])
                        nc.sync.dma_start(out=out[part, b, :, q0:q0 + n], in_=st[:, :n])

    _legalize_single_wait(nc)
    return nc


LAST_RESULT = None


def _ensure_axon_hooks_stub():
    """bass_utils imports antenv.axon_hooks when BASS_TRACE is set; the
    module is absent from this image.  Provide a no-op stub (get -> None)
    unless something already registered a real hook."""
    import sys
    import types

    if "antenv.axon_hooks" in sys.modules:
        return
    mod = types.ModuleType("antenv.axon_hooks")
    mod._hook = None
    mod.set_axon_ntff_profile_hook = lambda h: setattr(mod, "_hook", h)
    mod.get_axon_ntff_profile_hook = lambda: mod._hook
    sys.modules["antenv.axon_hooks"] = mod


def kernel(x_real, x_imag, k_real, k_imag, b_real, b_imag):
    global LAST_RESULT
    _ensure_axon_hooks_stub()
    from concourse.bass_utils import run_bass_kernel_spmd

    x_real = np.asarray(x_real, dtype=np.float32)
    x_imag = np.asarray(x_imag, dtype=np.float32)
    k_real = np.asarray(k_real, dtype=np.float32)
    k_imag = np.asarray(k_imag, dtype=np.float32)
    b_real = np.asarray(b_real, dtype=np.float32)
    b_imag = np.asarray(b_imag, dtype=np.float32)

    # ---- host-side input prep -------------------------------------------
    # padded channel-major image, channels = [x_real; x_imag]
    xp = np.zeros((B, ROWS, RS, 2 * CIN), np.float32)
    xp[:, 1:H + 1, 1:W + 1, :CIN] = x_real
    xp[:, 1:H + 1, 1:W + 1, CIN:] = x_imag
    xs_all = np.ascontiguousarray(xp.transpose(0, 3, 1, 2).reshape(B, 128, L)).astype(_BF16)

    # stacked weights: [ch, part, dy, dx, F] -> [128, 2304]
    wr = np.concatenate([k_real, -k_imag], axis=2)   # [3,3,128,F]
    wi = np.concatenate([k_imag, k_real], axis=2)
    kw = np.ascontiguousarray(
        np.stack([wr, wi]).transpose(3, 0, 1, 2, 4).reshape(128, 2 * 9 * F)
    ).astype(_BF16)

    if "nc" not in _CACHE:
        _CACHE["nc"] = _build_nc()
    nc = _CACHE["nc"]

    in_maps = [
        {"xs": xs_all[c * B_LOC:(c + 1) * B_LOC], "kw": kw} for c in range(N_CORES)
    ]
    res = None
    for attempt in range(3):
        try:
            res = run_bass_kernel_spmd(nc, in_maps, core_ids=list(range(N_CORES)))
            break
        except Exception:
            # transient device errors (e.g. NRT_EXEC_UNIT_UNRECOVERABLE) do
            # happen; retry before giving up
            if attempt == 2:
                raise
            import time as _time

            _time.sleep(2.0)
    LAST_RESULT = res

    # ---- host-side gather / unshard -------------------------------------
    final = np.empty((2, B, H, W, F), np.float32)
    for c in range(N_CORES):
        oc = res.results[c]["out"]                       # [2, B_LOC, F, NQ]
        v = oc.reshape(2, B_LOC, F, H, RS)[..., :W]
        final[:, c * B_LOC:(c + 1) * B_LOC] = v.transpose(0, 1, 3, 4, 2)

    if b_real.any():
        final[0] += b_real
    if b_imag.any():
        final[1] += b_imag
    return final


# revision 15
# speedup vs baseline: 1.0089x; 1.0080x over previous
"""ComplexConv2D Trainium2 kernel.

Reference computation (B=16, H=W=128, CIN=64, F=128, K=3, SAME, stride 1):
    real_out = conv(x_real, k_real) - conv(x_imag, k_imag) + b_real
    imag_out = conv(x_real, k_imag) + conv(x_imag, k_real) + b_imag
    return stack([real_out, imag_out])           # [2, B, H, W, F]

Strategy:
  * Data-parallel over batch: 2 images per NeuronCore x 8 cores.
  * Complex arithmetic is folded into the matmul contraction: stack
    [x_real; x_imag] channel-wise (K = 2*CIN = 128 = full PE width) and
    contract against stacked weights [k_real; -k_imag] (real part) and
    [k_imag; k_real] (imag part).  Each output part is then ONE ordinary
    3x3 conv with 128 input channels.
  * The conv is 9 shifted matmuls accumulated in PSUM.  The image lives in
    SBUF channel-major as [128ch, (H+4)*(W+2)] with a 1-pixel zero border;
    a tap (dy,dx) is just a free-dim slice offset dy*(W+2)+dx, so all 9
    taps stream from the same SBUF buffer with zero data movement.
  * Matmul: lhsT = weight tap [128ch, 128F] (stationary), rhs = image
    slice [128ch, 512pos] (moving), PSUM tile [128F, 512pos] fp32 = one
    bank.  9 accumulating matmuls per tile; 33 tiles cover one image.
  * bf16 inputs (host-cast), fp32 PSUM accumulation, fp32 output.
  * Output leaves the chip channel-major [F, positions]; the final
    transpose to NHWC plus removal of the 2 pad columns per row happens
    host-side during the gather.
"""

import os

import numpy as np
import ml_dtypes

B, H, W, CIN, F = 16, 128, 128, 64, 128
N_CORES = 8
B_LOC = B // N_CORES          # images per core
RS = W + 2                    # padded row stride (130)
ROWS = H + 3                  # 1 top pad + 1 bottom pad + 1 slack row (131)
L = ROWS * RS                 # flat padded image length (17030)
NQ_VALID = H * RS             # flat positions covering all valid outputs (16640)
NT = 33                       # output tiles per image: 32x512 + 1x256
NQ = NQ_VALID                 # flat output length on chip (16640)
TILE_N = [512] * 32 + [256]
TILE_Q0 = [512 * t for t in range(33)]

_BF16 = ml_dtypes.bfloat16

_CACHE = {}


def _legalize_single_wait(nc):
    """The pinned walrus build in this container accepts only a single
    sync-wait per instruction.  Tile attaches several waits to one
    instruction (drain, DMA, matmul...).  Hoist all-but-one wait onto
    fresh no-fuse NoOps on the same engine placed immediately before the
    instruction — same-engine program order preserves the AND semantics."""
    import concourse.mybir as mybir

    for f in nc.m.functions:
        for bb in f.blocks:
            newlist = []
            for inst in bb.instructions:
                si = inst.sync_info
                if si is not None and len(si.on_wait) > 1:
                    waits = list(si.on_wait)
                    del si.on_wait[:]
                    si.on_wait.append(waits[-1])
                    for k, w in enumerate(waits[:-1]):
                        nop = mybir.InstNoOp(
                            name=f"{inst.name}.sw{k}",
                            opcode="NoOp",
                            engine=inst.engine,
                            bass_nofuse=True,
                            sync_info=mybir.SyncInfo(on_wait=[w], on_update=[]),
                        )
                        newlist.append(nop)
                newlist.append(inst)
            bb.instructions[:] = newlist


def _patch_minimal_tail():
    """Tile's kernel tail is drain + two all-engine EVSEM-butterfly barriers
    around the sem resets (~8 us).  The barriers only exist to order the
    Pool-issued sem resets after every engine's last instruction — but the
    drain's global-clock waits already prove all work (every engine tick and
    every DMA receipt) is complete, so issue the resets from SP right after
    the drain and skip the barriers entirely."""
    import concourse.tile as tile
    from concourse.bass import compact_to_ranges
    from concourse.vector_clock import ScopedClock

    if getattr(tile.TileContext._drain_and_barrier, "_minimal_tail", False):
        return

    def _drain_and_barrier(self, tick_clock, wait_clock):
        nc = self.nc
        drain_inst = nc.sync.drain()
        wait_clock.add_sem_waits(
            drain_inst.ins, ScopedClock({None: tick_clock.global_clock})
        )
        popped = nc._tile_sem_poison_stack.pop()
        assert popped is self._sem_poison
        sem_nums = sorted(s.num for s in self.sems.allocated().values())
        for r in compact_to_ranges(sem_nums):
            nc.sync.drain(semaphore_range=r)   # == gpsimd.dma_reset, SP-issued
            nc.sync.sem_clear(r)

    _drain_and_barrier._minimal_tail = True
    tile.TileContext._drain_and_barrier = _drain_and_barrier


def _build_nc():
    import concourse.bass as bass
    import concourse.mybir as mybir
    import concourse.tile as tile

    _patch_minimal_tail()

    nc = bass.Bass()
    xs = nc.declare_dram_parameter("xs", [B_LOC, 128, L], mybir.dt.bfloat16, isOutput=False)
    kw = nc.declare_dram_parameter("kw", [128, 2 * 9 * F], mybir.dt.bfloat16, isOutput=False)
    out = nc.declare_dram_parameter("out", [2, B_LOC, F, NQ], mybir.dt.float32, isOutput=True)

    # graduated chunk boundaries: small first chunks so the first matmuls
    # can start as soon as ~0.3 MB has landed, big chunks for efficiency
    CHUNKS = [0, 1040, 3120, 6500, 10010, 13520, L]

    with tile.TileContext(nc) as tc:
        with (
            tc.tile_pool(name="kw", bufs=1) as kwp,
            tc.tile_pool(name="img", bufs=2) as imgp,
            tc.tile_pool(name="psum", bufs=8, space="PSUM") as psp,
            tc.tile_pool(name="stage", bufs=4) as stp,
        ):
            kw_sb = kwp.tile([128, 2 * 9 * F], mybir.dt.bfloat16)
            # The first LDWEIGHTS gates the whole pipeline: load the first
            # three taps alone (96 KB) so their completion isn't delayed by
            # image traffic, then the rest of part 0; part-1 weights aren't
            # needed until ~half way through image 0.
            nc.sync.dma_start(out=kw_sb[:, :3 * F], in_=kw[:, :3 * F])

            first = True
            for b in range(B_LOC):
                img = imgp.tile([128, L], mybir.dt.bfloat16)
                for ci, (c0, c1) in enumerate(zip(CHUNKS, CHUNKS[1:])):
                    nc.sync.dma_start(out=img[:, c0:c1], in_=xs[b, :, c0:c1])
                    if first and ci == 0:
                        # rest of part-0 weights ride behind the first chunk
                        nc.sync.dma_start(out=kw_sb[:, 3 * F:9 * F], in_=kw[:, 3 * F:9 * F])
                if first:
                    nc.sync.dma_start(out=kw_sb[:, 9 * F:], in_=kw[:, 9 * F:])
                    first = False
                for part in range(2):
                    for t in range(NT):
                        q0, n = TILE_Q0[t], TILE_N[t]
                        ps = psp.tile([128, 512], mybir.dt.float32)
                        for tap in range(9):
                            dy, dx = divmod(tap, 3)
                            off = q0 + dy * RS + dx
                            nc.tensor.matmul(
                                ps[:, :n],
                                kw_sb[:, (part * 9 + tap) * F:(part * 9 + tap + 1) * F],
                                img[:, off:off + n],
                                start=(tap == 0),
                                stop=(tap == 8),
                            )
                        st = stp.tile([128, 512], mybir.dt.float32)
                        nc.vector.tensor_copy(st[:, :n], ps[:, :n])
                        nc.sync.dma_start(out=out[part, b, :, q0:q0 + n], in_=st[:, :n])

    _legalize_single_wait(nc)
    return nc


LAST_RESULT = None


def _ensure_axon_hooks_stub():
    """bass_utils imports antenv.axon_hooks when BASS_TRACE is set; the
    module is absent from this image.  Provide a no-op stub (get -> None)
    unless something already registered a real hook."""
    import sys
    import types

    if "antenv.axon_hooks" in sys.modules:
        return
    mod = types.ModuleType("antenv.axon_hooks")
    mod._hook = None
    mod.set_axon_ntff_profile_hook = lambda h: setattr(mod, "_hook", h)
    mod.get_axon_ntff_profile_hook = lambda: mod._hook
    sys.modules["antenv.axon_hooks"] = mod


def kernel(x_real, x_imag, k_real, k_imag, b_real, b_imag):
    global LAST_RESULT
    _ensure_axon_hooks_stub()
    from concourse.bass_utils import run_bass_kernel_spmd

    x_real = np.asarray(x_real, dtype=np.float32)
    x_imag = np.asarray(x_imag, dtype=np.float32)
    k_real = np.asarray(k_real, dtype=np.float32)
    k_imag = np.asarray(k_imag, dtype=np.float32)
    b_real = np.asarray(b_real, dtype=np.float32)
    b_imag = np.asarray(b_imag, dtype=np.float32)

    # ---- host-side input prep -------------------------------------------
    # padded channel-major image, channels = [x_real; x_imag]
    xp = np.zeros((B, ROWS, RS, 2 * CIN), np.float32)
    xp[:, 1:H + 1, 1:W + 1, :CIN] = x_real
    xp[:, 1:H + 1, 1:W + 1, CIN:] = x_imag
    xs_all = np.ascontiguousarray(xp.transpose(0, 3, 1, 2).reshape(B, 128, L)).astype(_BF16)

    # stacked weights: [ch, part, dy, dx, F] -> [128, 2304]
    wr = np.concatenate([k_real, -k_imag], axis=2)   # [3,3,128,F]
    wi = np.concatenate([k_imag, k_real], axis=2)
    kw = np.ascontiguousarray(
        np.stack([wr, wi]).transpose(3, 0, 1, 2, 4).reshape(128, 2 * 9 * F)
    ).astype(_BF16)

    if "nc" not in _CACHE:
        _CACHE["nc"] = _build_nc()
    nc = _CACHE["nc"]

    in_maps = [
        {"xs": xs_all[c * B_LOC:(c + 1) * B_LOC], "kw": kw} for c in range(N_CORES)
    ]
    res = None
    for attempt in range(3):
        try:
            res = run_bass_kernel_spmd(nc, in_maps, core_ids=list(range(N_CORES)))
            break
        except Exception:
            # transient device errors (e.g. NRT_EXEC_UNIT_UNRECOVERABLE) do
            # happen; retry before giving up
            if attempt == 2:
                raise
            import time as _time

            _time.sleep(2.0)
    LAST_RESULT = res

    # ---- host-side gather / unshard -------------------------------------
    final = np.empty((2, B, H, W, F), np.float32)
    for c in range(N_CORES):
        oc = res.results[c]["out"]                       # [2, B_LOC, F, NQ]
        v = oc.reshape(2, B_LOC, F, H, RS)[..., :W]
        final[:, c * B_LOC:(c + 1) * B_LOC] = v.transpose(0, 1, 3, 4, 2)

    if b_real.any():
        final[0] += b_real
    if b_imag.any():
        final[1] += b_imag
    return final


# revision 16
# speedup vs baseline: 1.0103x; 1.0013x over previous
"""ComplexConv2D Trainium2 kernel.

Reference computation (B=16, H=W=128, CIN=64, F=128, K=3, SAME, stride 1):
    real_out = conv(x_real, k_real) - conv(x_imag, k_imag) + b_real
    imag_out = conv(x_real, k_imag) + conv(x_imag, k_real) + b_imag
    return stack([real_out, imag_out])           # [2, B, H, W, F]

Strategy:
  * Data-parallel over batch: 2 images per NeuronCore x 8 cores.
  * Complex arithmetic is folded into the matmul contraction: stack
    [x_real; x_imag] channel-wise (K = 2*CIN = 128 = full PE width) and
    contract against stacked weights [k_real; -k_imag] (real part) and
    [k_imag; k_real] (imag part).  Each output part is then ONE ordinary
    3x3 conv with 128 input channels.
  * The conv is 9 shifted matmuls accumulated in PSUM.  The image lives in
    SBUF channel-major as [128ch, (H+4)*(W+2)] with a 1-pixel zero border;
    a tap (dy,dx) is just a free-dim slice offset dy*(W+2)+dx, so all 9
    taps stream from the same SBUF buffer with zero data movement.
  * Matmul: lhsT = weight tap [128ch, 128F] (stationary), rhs = image
    slice [128ch, 512pos] (moving), PSUM tile [128F, 512pos] fp32 = one
    bank.  9 accumulating matmuls per tile; 33 tiles cover one image.
  * bf16 inputs (host-cast), fp32 PSUM accumulation, fp32 output.
  * Output leaves the chip channel-major [F, positions]; the final
    transpose to NHWC plus removal of the 2 pad columns per row happens
    host-side during the gather.
"""

import os

import numpy as np
import ml_dtypes

B, H, W, CIN, F = 16, 128, 128, 64, 128
N_CORES = 8
B_LOC = B // N_CORES          # images per core
RS = W + 2                    # padded row stride (130)
ROWS = H + 3                  # 1 top pad + 1 bottom pad + 1 slack row (131)
L = ROWS * RS                 # flat padded image length (17030)
NQ_VALID = H * RS             # flat positions covering all valid outputs (16640)
NT = 33                       # output tiles per image: 32x512 + 1x256
NQ = NQ_VALID                 # flat output length on chip (16640)
TILE_N = [512] * 32 + [256]
TILE_Q0 = [512 * t for t in range(33)]

_BF16 = ml_dtypes.bfloat16

_CACHE = {}


def _legalize_single_wait(nc):
    """The pinned walrus build in this container accepts only a single
    sync-wait per instruction.  Tile attaches several waits to one
    instruction (drain, DMA, matmul...).  Hoist all-but-one wait onto
    fresh no-fuse NoOps on the same engine placed immediately before the
    instruction — same-engine program order preserves the AND semantics."""
    import concourse.mybir as mybir

    for f in nc.m.functions:
        for bb in f.blocks:
            newlist = []
            for inst in bb.instructions:
                si = inst.sync_info
                if si is not None and len(si.on_wait) > 1:
                    waits = list(si.on_wait)
                    del si.on_wait[:]
                    si.on_wait.append(waits[-1])
                    for k, w in enumerate(waits[:-1]):
                        nop = mybir.InstNoOp(
                            name=f"{inst.name}.sw{k}",
                            opcode="NoOp",
                            engine=inst.engine,
                            bass_nofuse=True,
                            sync_info=mybir.SyncInfo(on_wait=[w], on_update=[]),
                        )
                        newlist.append(nop)
                newlist.append(inst)
            bb.instructions[:] = newlist


def _patch_minimal_tail():
    """Tile's kernel tail is drain + two all-engine EVSEM-butterfly barriers
    around the sem resets (~8 us).  The barriers only exist to order the
    Pool-issued sem resets after every engine's last instruction — but the
    drain's global-clock waits already prove all work (every engine tick and
    every DMA receipt) is complete, so issue the resets from SP right after
    the drain and skip the barriers entirely."""
    import concourse.tile as tile
    from concourse.bass import compact_to_ranges
    from concourse.vector_clock import ScopedClock

    if getattr(tile.TileContext._drain_and_barrier, "_minimal_tail", False):
        return

    def _drain_and_barrier(self, tick_clock, wait_clock):
        nc = self.nc
        drain_inst = nc.sync.drain()
        wait_clock.add_sem_waits(
            drain_inst.ins, ScopedClock({None: tick_clock.global_clock})
        )
        popped = nc._tile_sem_poison_stack.pop()
        assert popped is self._sem_poison
        sem_nums = sorted(s.num for s in self.sems.allocated().values())
        for r in compact_to_ranges(sem_nums):
            nc.sync.drain(semaphore_range=r)   # == gpsimd.dma_reset, SP-issued
            nc.sync.sem_clear(r)

    _drain_and_barrier._minimal_tail = True
    tile.TileContext._drain_and_barrier = _drain_and_barrier


def _build_nc():
    import concourse.bass as bass
    import concourse.mybir as mybir
    import concourse.tile as tile

    _patch_minimal_tail()

    nc = bass.Bass()
    xs = nc.declare_dram_parameter("xs", [B_LOC, 128, L], mybir.dt.bfloat16, isOutput=False)
    kw = nc.declare_dram_parameter("kw", [128, 2 * 9 * F], mybir.dt.bfloat16, isOutput=False)
    out = nc.declare_dram_parameter("out", [2, B_LOC, F, NQ], mybir.dt.float32, isOutput=True)

    # graduated chunk boundaries: small first chunks so the first matmuls
    # can start as soon as ~0.3 MB has landed, big chunks for efficiency
    CHUNKS = [0, 780, 2080, 4680, 8450, 12220, L]

    with tile.TileContext(nc) as tc:
        with (
            tc.tile_pool(name="kw", bufs=1) as kwp,
            tc.tile_pool(name="img", bufs=2) as imgp,
            tc.tile_pool(name="psum", bufs=8, space="PSUM") as psp,
            tc.tile_pool(name="stage", bufs=4) as stp,
        ):
            kw_sb = kwp.tile([128, 2 * 9 * F], mybir.dt.bfloat16)
            # The first LDWEIGHTS gates the whole pipeline: load the first
            # three taps alone (96 KB) so their completion isn't delayed by
            # image traffic, then the rest of part 0; part-1 weights aren't
            # needed until ~half way through image 0.
            nc.sync.dma_start(out=kw_sb[:, :3 * F], in_=kw[:, :3 * F])

            first = True
            for b in range(B_LOC):
                img = imgp.tile([128, L], mybir.dt.bfloat16)
                for ci, (c0, c1) in enumerate(zip(CHUNKS, CHUNKS[1:])):
                    nc.sync.dma_start(out=img[:, c0:c1], in_=xs[b, :, c0:c1])
                    if first and ci == 0:
                        # rest of part-0 weights ride behind the first chunk
                        nc.sync.dma_start(out=kw_sb[:, 3 * F:9 * F], in_=kw[:, 3 * F:9 * F])
                if first:
                    nc.sync.dma_start(out=kw_sb[:, 9 * F:], in_=kw[:, 9 * F:])
                    first = False
                for part in range(2):
                    for t in range(NT):
                        q0, n = TILE_Q0[t], TILE_N[t]
                        ps = psp.tile([128, 512], mybir.dt.float32)
                        for tap in range(9):
                            dy, dx = divmod(tap, 3)
                            off = q0 + dy * RS + dx
                            nc.tensor.matmul(
                                ps[:, :n],
                                kw_sb[:, (part * 9 + tap) * F:(part * 9 + tap + 1) * F],
                                img[:, off:off + n],
                                start=(tap == 0),
                                stop=(tap == 8),
                            )
                        st = stp.tile([128, 512], mybir.dt.float32)
                        nc.vector.tensor_copy(st[:, :n], ps[:, :n])
                        nc.sync.dma_start(out=out[part, b, :, q0:q0 + n], in_=st[:, :n])

    _legalize_single_wait(nc)
    return nc


LAST_RESULT = None


def _ensure_axon_hooks_stub():
    """bass_utils imports antenv.axon_hooks when BASS_TRACE is set; the
    module is absent from this image.  Provide a no-op stub (get -> None)
    unless something already registered a real hook."""
    import sys
    import types

    if "antenv.axon_hooks" in sys.modules:
        return
    mod = types.ModuleType("antenv.axon_hooks")
    mod._hook = None
    mod.set_axon_ntff_profile_hook = lambda h: setattr(mod, "_hook", h)
    mod.get_axon_ntff_profile_hook = lambda: mod._hook
    sys.modules["antenv.axon_hooks"] = mod


def kernel(x_real, x_imag, k_real, k_imag, b_real, b_imag):
    global LAST_RESULT
    _ensure_axon_hooks_stub()
    from concourse.bass_utils import run_bass_kernel_spmd

    x_real = np.asarray(x_real, dtype=np.float32)
    x_imag = np.asarray(x_imag, dtype=np.float32)
    k_real = np.asarray(k_real, dtype=np.float32)
    k_imag = np.asarray(k_imag, dtype=np.float32)
    b_real = np.asarray(b_real, dtype=np.float32)
    b_imag = np.asarray(b_imag, dtype=np.float32)

    # ---- host-side input prep -------------------------------------------
    # padded channel-major image, channels = [x_real; x_imag]
    xp = np.zeros((B, ROWS, RS, 2 * CIN), np.float32)
    xp[:, 1:H + 1, 1:W + 1, :CIN] = x_real
    xp[:, 1:H + 1, 1:W + 1, CIN:] = x_imag
    xs_all = np.ascontiguousarray(xp.transpose(0, 3, 1, 2).reshape(B, 128, L)).astype(_BF16)

    # stacked weights: [ch, part, dy, dx, F] -> [128, 2304]
    wr = np.concatenate([k_real, -k_imag], axis=2)   # [3,3,128,F]
    wi = np.concatenate([k_imag, k_real], axis=2)
    kw = np.ascontiguousarray(
        np.stack([wr, wi]).transpose(3, 0, 1, 2, 4).reshape(128, 2 * 9 * F)
    ).astype(_BF16)

    if "nc" not in _CACHE:
        _CACHE["nc"] = _build_nc()
    nc = _CACHE["nc"]

    in_maps = [
        {"xs": xs_all[c * B_LOC:(c + 1) * B_LOC], "kw": kw} for c in range(N_CORES)
    ]
    res = None
    for attempt in range(3):
        try:
            res = run_bass_kernel_spmd(nc, in_maps, core_ids=list(range(N_CORES)))
            break
        except Exception:
            # transient device errors (e.g. NRT_EXEC_UNIT_UNRECOVERABLE) do
            # happen; retry before giving up
            if attempt == 2:
                raise
            import time as _time

            _time.sleep(2.0)
    LAST_RESULT = res

    # ---- host-side gather / unshard -------------------------------------
    final = np.empty((2, B, H, W, F), np.float32)
    for c in range(N_CORES):
        oc = res.results[c]["out"]                       # [2, B_LOC, F, NQ]
        v = oc.reshape(2, B_LOC, F, H, RS)[..., :W]
        final[:, c * B_LOC:(c + 1) * B_LOC] = v.transpose(0, 1, 3, 4, 2)

    if b_real.any():
        final[0] += b_real
    if b_imag.any():
        final[1] += b_imag
    return final


# revision 18
# speedup vs baseline: 1.0240x; 1.0136x over previous
"""ComplexConv2D Trainium2 kernel.

Reference computation (B=16, H=W=128, CIN=64, F=128, K=3, SAME, stride 1):
    real_out = conv(x_real, k_real) - conv(x_imag, k_imag) + b_real
    imag_out = conv(x_real, k_imag) + conv(x_imag, k_real) + b_imag
    return stack([real_out, imag_out])           # [2, B, H, W, F]

Strategy:
  * Data-parallel over batch: 2 images per NeuronCore x 8 cores.
  * Complex arithmetic is folded into the matmul contraction: stack
    [x_real; x_imag] channel-wise (K = 2*CIN = 128 = full PE width) and
    contract against stacked weights [k_real; -k_imag] (real part) and
    [k_imag; k_real] (imag part).  Each output part is then ONE ordinary
    3x3 conv with 128 input channels.
  * The conv is 9 shifted matmuls accumulated in PSUM.  The image lives in
    SBUF channel-major as [128ch, (H+4)*(W+2)] with a 1-pixel zero border;
    a tap (dy,dx) is just a free-dim slice offset dy*(W+2)+dx, so all 9
    taps stream from the same SBUF buffer with zero data movement.
  * Matmul: lhsT = weight tap [128ch, 128F] (stationary), rhs = image
    slice [128ch, 512pos] (moving), PSUM tile [128F, 512pos] fp32 = one
    bank.  9 accumulating matmuls per tile; 33 tiles cover one image.
  * bf16 inputs (host-cast), fp32 PSUM accumulation, fp32 output.
  * Output leaves the chip channel-major [F, positions]; the final
    transpose to NHWC plus removal of the 2 pad columns per row happens
    host-side during the gather.
"""

import os

import numpy as np
import ml_dtypes

B, H, W, CIN, F = 16, 128, 128, 64, 128
N_CORES = 8
B_LOC = B // N_CORES          # images per core
RS = W + 2                    # padded row stride (130)
ROWS = H + 3                  # 1 top pad + 1 bottom pad + 1 slack row (131)
L = ROWS * RS                 # flat padded image length (17030)
NQ_VALID = H * RS             # flat positions covering all valid outputs (16640)
NT = 33                       # output tiles per image: 32x512 + 1x256
NQ = NQ_VALID                 # flat output length on chip (16640)
TILE_N = [512] * 32 + [256]
TILE_Q0 = [512 * t for t in range(33)]

_BF16 = ml_dtypes.bfloat16

_CACHE = {}


def _legalize_single_wait(nc):
    """The pinned walrus build in this container accepts only a single
    sync-wait per instruction.  Tile attaches several waits to one
    instruction (drain, DMA, matmul...).  Hoist all-but-one wait onto
    fresh no-fuse NoOps on the same engine placed immediately before the
    instruction — same-engine program order preserves the AND semantics."""
    import concourse.mybir as mybir

    # Drop the preamble memsets of framework const tensors nothing reads
    # (birverifier: "Non-output memory location with no reader") — they sit
    # on Pool's critical path to the init rendezvous.
    def _dead_const_memset(inst):
        if type(inst).__name__ != "InstMemset":
            return False
        try:
            return all(
                (o.memref or o.memsetref or "").startswith("const-")
                for o in inst.outs
            )
        except (AttributeError, TypeError):
            return False

    for f in nc.m.functions:
        for bb in f.blocks:
            newlist = []
            for inst in bb.instructions:
                if _dead_const_memset(inst):
                    continue
                si = inst.sync_info
                if si is not None and len(si.on_wait) > 1:
                    waits = list(si.on_wait)
                    del si.on_wait[:]
                    si.on_wait.append(waits[-1])
                    for k, w in enumerate(waits[:-1]):
                        nop = mybir.InstNoOp(
                            name=f"{inst.name}.sw{k}",
                            opcode="NoOp",
                            engine=inst.engine,
                            bass_nofuse=True,
                            sync_info=mybir.SyncInfo(on_wait=[w], on_update=[]),
                        )
                        newlist.append(nop)
                newlist.append(inst)
            bb.instructions[:] = newlist


def _patch_minimal_tail():
    """Tile's kernel tail is drain + two all-engine EVSEM-butterfly barriers
    around the sem resets (~8 us).  The barriers only exist to order the
    Pool-issued sem resets after every engine's last instruction — but the
    drain's global-clock waits already prove all work (every engine tick and
    every DMA receipt) is complete, so issue the resets from SP right after
    the drain and skip the barriers entirely."""
    import concourse.tile as tile
    from concourse.bass import compact_to_ranges
    from concourse.vector_clock import ScopedClock

    if getattr(tile.TileContext._drain_and_barrier, "_minimal_tail", False):
        return

    def _drain_and_barrier(self, tick_clock, wait_clock):
        nc = self.nc
        drain_inst = nc.sync.drain()
        wait_clock.add_sem_waits(
            drain_inst.ins, ScopedClock({None: tick_clock.global_clock})
        )
        popped = nc._tile_sem_poison_stack.pop()
        assert popped is self._sem_poison
        sem_nums = sorted(s.num for s in self.sems.allocated().values())
        for r in compact_to_ranges(sem_nums):
            nc.sync.drain(semaphore_range=r)   # == gpsimd.dma_reset, SP-issued
            nc.sync.sem_clear(r)

    _drain_and_barrier._minimal_tail = True
    tile.TileContext._drain_and_barrier = _drain_and_barrier


def _build_nc():
    import concourse.bass as bass
    import concourse.mybir as mybir
    import concourse.tile as tile

    _patch_minimal_tail()

    nc = bass.Bass()
    xs = nc.declare_dram_parameter("xs", [B_LOC, 128, L], mybir.dt.bfloat16, isOutput=False)
    kw = nc.declare_dram_parameter("kw", [128, 2 * 9 * F], mybir.dt.bfloat16, isOutput=False)
    out = nc.declare_dram_parameter("out", [2, B_LOC, F, NQ], mybir.dt.float32, isOutput=True)

    # graduated chunk boundaries: small first chunks so the first matmuls
    # can start as soon as ~0.3 MB has landed, big chunks for efficiency
    CHUNKS = [0, 780, 2080, 4680, 8450, 12220, L]

    with tile.TileContext(nc) as tc:
        with (
            tc.tile_pool(name="kw", bufs=1) as kwp,
            tc.tile_pool(name="img", bufs=2) as imgp,
            tc.tile_pool(name="psum", bufs=8, space="PSUM") as psp,
            tc.tile_pool(name="stage", bufs=4) as stp,
        ):
            kw_sb = kwp.tile([128, 2 * 9 * F], mybir.dt.bfloat16)
            # The first LDWEIGHTS gates the whole pipeline: load the first
            # three taps alone (96 KB) so their completion isn't delayed by
            # image traffic, then the rest of part 0; part-1 weights aren't
            # needed until ~half way through image 0.
            nc.sync.dma_start(out=kw_sb[:, :3 * F], in_=kw[:, :3 * F])

            first = True
            for b in range(B_LOC):
                img = imgp.tile([128, L], mybir.dt.bfloat16)
                for ci, (c0, c1) in enumerate(zip(CHUNKS, CHUNKS[1:])):
                    nc.sync.dma_start(out=img[:, c0:c1], in_=xs[b, :, c0:c1])
                    if first and ci == 0:
                        # rest of part-0 weights ride behind the first chunk
                        nc.sync.dma_start(out=kw_sb[:, 3 * F:9 * F], in_=kw[:, 3 * F:9 * F])
                if first:
                    nc.sync.dma_start(out=kw_sb[:, 9 * F:], in_=kw[:, 9 * F:])
                    first = False
                for part in range(2):
                    for t in range(NT):
                        q0, n = TILE_Q0[t], TILE_N[t]
                        ps = psp.tile([128, 512], mybir.dt.float32)
                        for tap in range(9):
                            dy, dx = divmod(tap, 3)
                            off = q0 + dy * RS + dx
                            nc.tensor.matmul(
                                ps[:, :n],
                                kw_sb[:, (part * 9 + tap) * F:(part * 9 + tap + 1) * F],
                                img[:, off:off + n],
                                start=(tap == 0),
                                stop=(tap == 8),
                            )
                        st = stp.tile([128, 512], mybir.dt.float32)
                        nc.vector.tensor_copy(st[:, :n], ps[:, :n])
                        nc.sync.dma_start(out=out[part, b, :, q0:q0 + n], in_=st[:, :n])

    _legalize_single_wait(nc)
    return nc


LAST_RESULT = None


def _ensure_axon_hooks_stub():
    """bass_utils imports antenv.axon_hooks when BASS_TRACE is set; the
    module is absent from this image.  Provide a no-op stub (get -> None)
    unless something already registered a real hook."""
    import sys
    import types

    if "antenv.axon_hooks" in sys.modules:
        return
    mod = types.ModuleType("antenv.axon_hooks")
    mod._hook = None
    mod.set_axon_ntff_profile_hook = lambda h: setattr(mod, "_hook", h)
    mod.get_axon_ntff_profile_hook = lambda: mod._hook
    sys.modules["antenv.axon_hooks"] = mod


def kernel(x_real, x_imag, k_real, k_imag, b_real, b_imag):
    global LAST_RESULT
    _ensure_axon_hooks_stub()
    from concourse.bass_utils import run_bass_kernel_spmd

    x_real = np.asarray(x_real, dtype=np.float32)
    x_imag = np.asarray(x_imag, dtype=np.float32)
    k_real = np.asarray(k_real, dtype=np.float32)
    k_imag = np.asarray(k_imag, dtype=np.float32)
    b_real = np.asarray(b_real, dtype=np.float32)
    b_imag = np.asarray(b_imag, dtype=np.float32)

    # ---- host-side input prep -------------------------------------------
    # padded channel-major image, channels = [x_real; x_imag]
    xp = np.zeros((B, ROWS, RS, 2 * CIN), np.float32)
    xp[:, 1:H + 1, 1:W + 1, :CIN] = x_real
    xp[:, 1:H + 1, 1:W + 1, CIN:] = x_imag
    xs_all = np.ascontiguousarray(xp.transpose(0, 3, 1, 2).reshape(B, 128, L)).astype(_BF16)

    # stacked weights: [ch, part, dy, dx, F] -> [128, 2304]
    wr = np.concatenate([k_real, -k_imag], axis=2)   # [3,3,128,F]
    wi = np.concatenate([k_imag, k_real], axis=2)
    kw = np.ascontiguousarray(
        np.stack([wr, wi]).transpose(3, 0, 1, 2, 4).reshape(128, 2 * 9 * F)
    ).astype(_BF16)

    if "nc" not in _CACHE:
        _CACHE["nc"] = _build_nc()
    nc = _CACHE["nc"]

    in_maps = [
        {"xs": xs_all[c * B_LOC:(c + 1) * B_LOC], "kw": kw} for c in range(N_CORES)
    ]
    res = None
    for attempt in range(3):
        try:
            res = run_bass_kernel_spmd(nc, in_maps, core_ids=list(range(N_CORES)))
            break
        except Exception:
            # transient device errors (e.g. NRT_EXEC_UNIT_UNRECOVERABLE) do
            # happen; retry before giving up
            if attempt == 2:
                raise
            import time as _time

            _time.sleep(2.0)
    LAST_RESULT = res

    # ---- host-side gather / unshard -------------------------------------
    final = np.empty((2, B, H, W, F), np.float32)
    for c in range(N_CORES):
        oc = res.results[c]["out"]                       # [2, B_LOC, F, NQ]
        v = oc.reshape(2, B_LOC, F, H, RS)[..., :W]
        final[:, c * B_LOC:(c + 1) * B_LOC] = v.transpose(0, 1, 3, 4, 2)

    if b_real.any():
        final[0] += b_real
    if b_imag.any():
        final[1] += b_imag
    return final
